# revision 2
# baseline (speedup 1.0000x reference)
import numpy as np

# Problem (hardcoded from spec/reference):
#   x [131072,3]; per-cartesian-AO params: centers_ao [240,3], ls [240,3] int32,
#   anorms [240], coeffs/zetas [240,6], normalization [240], cart2sph [240,224].
#   Output [131072,224] float32.
#   Structure: 16 atoms x shells [s,s,s,p,p,d] -> 96 shells, 240 cart AOs,
#   6 primitives per shell. Per-AO arrays are duplicated per shell/atom.
#
# Strategy (8 NeuronCores, pure data parallel over points):
#   Host precomputes per-point features xf[26] = [1, x, y, z, xx, xy, xz, yy,
#   yz, zz, r2_atom0..r2_atom15] (r2 exact in fp32 on host). On device, in a
#   transposed layout (points along the free dim, F=512 per chunk):
#     m1 (PE, float32r): W1[26,816]^T @ xf -> [Arg(576) ; ang(240)] where
#         Arg[(s,p)] = -zeta[s,p]*r2[atom(s)] and ang[a] = cart angular poly.
#     exp (ACT):   E = exp(Arg) -> bf16 SBUF.
#     m2 (PE bf16): rad[96] = Coef[576,96]^T @ E  (contract 6 primitives).
#     m3 (PE bf16): rad_ao[240] = Expand[96,240]^T @ rad (shell->AO).
#     G (DVE): G = ang * rad_ao -> bf16.
#     m4 (PE bf16): out[128pts,224] = G[:,blk]^T @ C2 with C2 = diag(anorm*
#         normalization) @ cart2sph; output written per 128-point block in
#         row-major order, copied PSUM->SBUF on ACT, DMA'd to DRAM.
#   No collectives; each core computes its own 16384-point slice.

N_CORES = 8
N_POINTS = 131072
NPC = N_POINTS // N_CORES  # 16384
FCHUNK = 512
NATOM = 16
SHELL_LS_PER_ATOM = [0, 0, 0, 1, 1, 2]
NCART_OF_L = {0: 1, 1: 3, 2: 6}
NSH = NATOM * len(SHELL_LS_PER_ATOM)  # 96
NAO = 240
NSPH = 224
NPRIM = 6
NARG = NSH * NPRIM  # 576
NFEAT = 10 + NATOM  # 26
M1COLS = NARG + NAO  # 816

# AO/shell bookkeeping (reference order: per atom, per shell, per cartesian).
_AO_SHELL = []
_SHELL_ATOM = []
_s = 0
for _a in range(NATOM):
    for _l in SHELL_LS_PER_ATOM:
        _AO_SHELL.extend([_s] * NCART_OF_L[_l])
        _SHELL_ATOM.append(_a)
        _s += 1
_AO_SHELL = np.asarray(_AO_SHELL)
_SHELL_ATOM = np.asarray(_SHELL_ATOM)
_FIRST_AO_OF_SHELL = np.searchsorted(_AO_SHELL, np.arange(NSH))

_MON_IDX = {
    (): 0, (0,): 1, (1,): 2, (2,): 3,
    (0, 0): 4, (0, 1): 5, (0, 2): 6, (1, 1): 7, (1, 2): 8, (2, 2): 9,
}

_STATE = {}


def _expand_ao_poly(lvec, c):
    """Coefficients of prod_k (x_k - c_k)^l_k in the 10-monomial basis."""
    terms = {(): 1.0}
    for k in range(3):
        l = int(lvec[k])
        if l == 0:
            axis = {(): 1.0}
        elif l == 1:
            axis = {(k,): 1.0, (): -float(c[k])}
        elif l == 2:
            axis = {(k, k): 1.0, (k,): -2.0 * float(c[k]), (): float(c[k]) ** 2}
        else:
            raise ValueError(f"unsupported l={l}")
        new = {}
        for m1, c1 in terms.items():
            for m2, c2 in axis.items():
                m = tuple(sorted(m1 + m2))
                new[m] = new.get(m, 0.0) + c1 * c2
        terms = new
    return terms


def _build_constants(centers_ao, ls, anorms, coeffs, zetas, normalization, cart2sph):
    import ml_dtypes

    first_ao_atom = np.arange(NATOM) * (NAO // NATOM)
    centers_atom = centers_ao[first_ao_atom]              # [16,3]
    zetas_sh = zetas[_FIRST_AO_OF_SHELL]                  # [96,6]
    coeffs_sh = coeffs[_FIRST_AO_OF_SHELL]                # [96,6]

    # Sanity-check the assumed duplication structure; caller falls back if not.
    ok = (
        np.array_equal(zetas, zetas_sh[_AO_SHELL])
        and np.array_equal(coeffs, coeffs_sh[_AO_SHELL])
        and np.array_equal(centers_ao, centers_atom[_SHELL_ATOM[_AO_SHELL]])
        and int(ls.sum(axis=1).max()) <= 2
    )

    w1 = np.zeros((NFEAT, M1COLS), np.float32)
    for s in range(NSH):
        for p in range(NPRIM):
            w1[10 + s // 6, s * NPRIM + p] = -zetas_sh[s, p]
    for a in range(NAO):
        for mon, cf in _expand_ao_poly(ls[a], centers_ao[a]).items():
            w1[_MON_IDX[mon], NARG + a] = cf

    coefm = np.zeros((NARG, NSH), np.float32)
    for s in range(NSH):
        coefm[s * NPRIM:(s + 1) * NPRIM, s] = coeffs_sh[s]

    expm = np.zeros((NSH, NAO), np.float32)
    expm[_AO_SHELL, np.arange(NAO)] = 1.0

    c2 = (anorms * normalization)[:, None] * cart2sph     # [240,224]

    bf16 = ml_dtypes.bfloat16
    return ok, {
        "w1": w1,
        "coef": coefm.astype(bf16),
        "expm": expm.astype(bf16),
        "c2": c2.astype(bf16),
        "centers_atom": centers_atom,
    }


def _build_features(x, centers_atom):
    """xf [26, N]: [1, x, y, z, xx, xy, xz, yy, yz, zz, r2_0..r2_15]."""
    n = x.shape[0]
    xf = np.empty((NFEAT, n), np.float32)
    xf[0] = 1.0
    xf[1:4] = x.T
    xf[4] = x[:, 0] * x[:, 0]
    xf[5] = x[:, 0] * x[:, 1]
    xf[6] = x[:, 0] * x[:, 2]
    xf[7] = x[:, 1] * x[:, 1]
    xf[8] = x[:, 1] * x[:, 2]
    xf[9] = x[:, 2] * x[:, 2]
    dx = x[:, None, :] - centers_atom[None, :, :]
    xf[10:] = np.einsum("nak,nak->na", dx, dx).T
    return xf


def build_module(npc=NPC, fchunk=FCHUNK):
    """Build the per-core Bass/Tile module (same program on all cores)."""
    from contextlib import ExitStack

    import concourse.bass as bass
    import concourse.tile as tile
    from concourse import bacc, mybir

    dt = mybir.dt
    Exp = mybir.ActivationFunctionType.Exp
    F = fchunk
    nchunks = npc // F
    assert npc % F == 0 and F % 128 == 0

    nc = bacc.Bacc("TRN2", target_bir_lowering=False, debug=False)
    xf = nc.dram_tensor("xf", [NFEAT, npc], dt.float32r, kind="ExternalInput").ap()
    w1 = nc.dram_tensor("w1", [NFEAT, M1COLS], dt.float32r, kind="ExternalInput").ap()
    coef = nc.dram_tensor("coef", [NARG, NSH], dt.bfloat16, kind="ExternalInput").ap()
    expm = nc.dram_tensor("expm", [NSH, NAO], dt.bfloat16, kind="ExternalInput").ap()
    c2 = nc.dram_tensor("c2", [NAO, NSPH], dt.bfloat16, kind="ExternalInput").ap()
    out = nc.dram_tensor("out", [npc, NSPH], dt.float32, kind="ExternalOutput").ap()

    # m1 output column blocks: 5 Arg tiles then 2 ang tiles.
    col_blocks = [(0, 128), (128, 256), (256, 384), (384, 512), (512, 576),
                  (576, 704), (704, 816)]

    with tile.TileContext(nc) as tc, ExitStack() as ctx:
        consts = ctx.enter_context(tc.tile_pool(name="consts", bufs=1))
        xf_pool = ctx.enter_context(tc.tile_pool(name="xfp", bufs=3))
        e_pool = ctx.enter_context(tc.tile_pool(name="ep", bufs=10))
        ang_pool = ctx.enter_context(tc.tile_pool(name="angp", bufs=4))
        radsb_pool = ctx.enter_context(tc.tile_pool(name="radsbp", bufs=2))
        g_pool = ctx.enter_context(tc.tile_pool(name="gp", bufs=4))
        osb_pool = ctx.enter_context(tc.tile_pool(name="osbp", bufs=4))
        argang_ps = ctx.enter_context(tc.tile_pool(name="argangps", bufs=3, space="PSUM"))
        rad_ps = ctx.enter_context(tc.tile_pool(name="radps", bufs=1, space="PSUM"))
        radao_ps = ctx.enter_context(tc.tile_pool(name="radaops", bufs=2, space="PSUM"))
        out_ps = ctx.enter_context(tc.tile_pool(name="outps", bufs=2, space="PSUM"))

        w1_sb = consts.tile([NFEAT, M1COLS], dt.float32r, tag="w1")
        nc.sync.dma_start(out=w1_sb, in_=w1)
        coef_sb = []
        for t in range(5):
            m = min(128, NARG - t * 128)
            ct = consts.tile([m, NSH], dt.bfloat16, tag=f"coef{t}")
            nc.sync.dma_start(out=ct, in_=coef[t * 128:t * 128 + m, :])
            coef_sb.append((ct, m))
        expm_sb = consts.tile([NSH, NAO], dt.bfloat16, tag="expm")
        nc.sync.dma_start(out=expm_sb, in_=expm)
        c2_hi = consts.tile([128, NSPH], dt.bfloat16, tag="c2hi")
        nc.sync.dma_start(out=c2_hi, in_=c2[0:128, :])
        c2_lo = consts.tile([NAO - 128, NSPH], dt.bfloat16, tag="c2lo")
        nc.sync.dma_start(out=c2_lo, in_=c2[128:NAO, :])

        for ci in range(nchunks):
            xs = xf_pool.tile([NFEAT, F], dt.float32r, tag="xs")
            nc.sync.dma_start(out=xs, in_=xf[:, ci * F:(ci + 1) * F])

            e_list, ang_list = [], []
            for (c0, c1) in col_blocks:
                m = c1 - c0
                ps = argang_ps.tile([128, F], dt.float32, tag="argang")
                nc.tensor.matmul(out=ps[:m], lhsT=w1_sb[:, c0:c1], rhs=xs,
                                 start=True, stop=True)
                if c0 < NARG:
                    et = e_pool.tile([128, F], dt.bfloat16, tag="e")
                    nc.scalar.activation(out=et[:m], in_=ps[:m], func=Exp)
                    e_list.append((et, m))
                else:
                    at = ang_pool.tile([128, F], dt.bfloat16, tag="ang")
                    nc.vector.tensor_copy(out=at[:m], in_=ps[:m])
                    ang_list.append((at, m))

            rad = rad_ps.tile([NSH, F], dt.float32, tag="rad")
            for t, (et, m) in enumerate(e_list):
                nc.tensor.matmul(out=rad, lhsT=coef_sb[t][0], rhs=et[:m],
                                 start=(t == 0), stop=(t == len(e_list) - 1))
            rad_sb = radsb_pool.tile([NSH, F], dt.bfloat16, tag="radsb")
            nc.vector.tensor_copy(out=rad_sb, in_=rad)

            g_list = []
            for h, (a0, a1) in enumerate([(0, 128), (128, NAO)]):
                m = a1 - a0
                rao = radao_ps.tile([128, F], dt.float32, tag="radao")
                nc.tensor.matmul(out=rao[:m], lhsT=expm_sb[:, a0:a1], rhs=rad_sb,
                                 start=True, stop=True)
                gt = g_pool.tile([128, F], dt.bfloat16, tag="g")
                at, am = ang_list[h]
                assert am == m
                nc.vector.tensor_mul(gt[:m], rao[:m], at[:m])
                g_list.append((gt, m))

            for b in range(F // 128):
                ops = out_ps.tile([128, NSPH], dt.float32, tag="ops")
                nc.tensor.matmul(out=ops, lhsT=g_list[0][0][:, b * 128:(b + 1) * 128],
                                 rhs=c2_hi, start=True, stop=False)
                nc.tensor.matmul(out=ops,
                                 lhsT=g_list[1][0][:g_list[1][1], b * 128:(b + 1) * 128],
                                 rhs=c2_lo, start=False, stop=True)
                osb = osb_pool.tile([128, NSPH], dt.float32, tag="osb")
                nc.scalar.copy(out=osb, in_=ops)
                r0 = ci * F + b * 128
                nc.sync.dma_start(out=out[r0:r0 + 128, :], in_=osb)

    nc.compile()
    return nc


class _Runner:
    """Caches the jitted shard_map(bass_exec) across kernel() calls."""

    def __init__(self, nc, n_cores):
        import jax
        import concourse.mybir as mybir
        from concourse import bass2jax
        from jax.experimental.shard_map import shard_map
        from jax.sharding import Mesh, PartitionSpec

        bass2jax.install_neuronx_cc_hook()
        self.nc = nc
        self.n_cores = n_cores

        in_names, out_names, out_avals = [], [], []
        for alloc in nc.m.functions[0].allocations:
            if not isinstance(alloc, mybir.MemoryLocationSet):
                continue
            name = alloc.memorylocations[0].name
            if alloc.kind == "ExternalInput":
                in_names.append(name)
            elif alloc.kind == "ExternalOutput":
                out_names.append(name)
                out_avals.append(jax.core.ShapedArray(
                    tuple(alloc.tensor_shape), mybir.dt.np(alloc.dtype)))
        self.in_names = list(in_names)
        self.out_names = out_names
        self.out_avals = out_avals
        n_params = len(in_names)
        all_names = in_names + out_names

        def _body(*args):
            outs = bass2jax._bass_exec_p.bind(
                *args,
                out_avals=tuple(out_avals),
                in_names=tuple(all_names),
                out_names=tuple(out_names),
                lowering_input_output_aliases=(),
                sim_require_finite=True,
                sim_require_nnan=True,
                nc=nc,
            )
            return tuple(outs)

        devices = jax.devices()[:n_cores]
        assert len(devices) == n_cores
        mesh = Mesh(np.asarray(devices), ("core",))
        nargs = n_params + len(out_names)
        self._fn = jax.jit(
            shard_map(_body, mesh=mesh,
                      in_specs=(PartitionSpec("core"),) * nargs,
                      out_specs=(PartitionSpec("core"),) * len(out_names),
                      check_rep=False),
            donate_argnums=tuple(range(n_params, nargs)),
            keep_unused=True,
        )

    def __call__(self, global_inputs):
        """global_inputs: dict name -> [n_cores*dim0, ...] array. Returns
        list of global outputs (concatenated along axis 0)."""
        args = [global_inputs[name] for name in self.in_names]
        zeros = [np.zeros((self.n_cores * av.shape[0], *av.shape[1:]), av.dtype)
                 for av in self.out_avals]
        outs = self._fn(*args, *zeros)
        return [np.asarray(o) for o in outs]


def _np_compute(x, centers_ao, ls, anorms, coeffs, zetas, normalization, cart2sph):
    # CPU fallback, chunked over points to bound memory.
    N = x.shape[0]
    S = cart2sph.shape[1]
    out = np.empty((N, S), dtype=np.float32)
    w = (anorms * normalization).astype(np.float32)
    step = 8192
    for i in range(0, N, step):
        xb = x[i:i + step]
        dx = xb[:, None, :] - centers_ao[None, :, :]
        r2 = np.sum(dx * dx, axis=-1)
        ang = np.ones(r2.shape, dtype=np.float32)
        for k in range(3):
            d = dx[..., k]
            l = ls[None, :, k]
            ang = ang * np.where(l == 0, 1.0, np.where(l == 1, d, d * d)).astype(np.float32)
        rad = np.sum(coeffs[None] * np.exp(-zetas[None] * r2[..., None]), axis=-1)
        phi = (w[None] * ang * rad).astype(np.float32)
        out[i:i + step] = phi @ cart2sph
    return out


def _global_inputs(x, consts_np):
    """Assemble the concatenated per-core input arrays."""
    xf = _build_features(x, consts_np["centers_atom"])       # [26, N]
    xf_g = np.ascontiguousarray(
        xf.reshape(NFEAT, N_CORES, NPC).transpose(1, 0, 2).reshape(N_CORES * NFEAT, NPC))
    return {
        "xf": xf_g,
        "w1": np.tile(consts_np["w1"], (N_CORES, 1)),
        "coef": np.tile(consts_np["coef"], (N_CORES, 1)),
        "expm": np.tile(consts_np["expm"], (N_CORES, 1)),
        "c2": np.tile(consts_np["c2"], (N_CORES, 1)),
    }


def kernel(**inputs):
    x = np.asarray(inputs["x"], dtype=np.float32)
    centers_ao = np.asarray(inputs["centers_ao"], dtype=np.float32)
    ls = np.asarray(inputs["ls"], dtype=np.int32)
    anorms = np.asarray(inputs["anorms"], dtype=np.float32)
    coeffs = np.asarray(inputs["coeffs"], dtype=np.float32)
    zetas = np.asarray(inputs["zetas"], dtype=np.float32)
    normalization = np.asarray(inputs["normalization"], dtype=np.float32)
    cart2sph = np.asarray(inputs["cart2sph"], dtype=np.float32)

    try:
        if x.shape != (N_POINTS, 3) or cart2sph.shape != (NAO, NSPH):
            raise RuntimeError("unexpected shapes")
        ok, consts_np = _build_constants(
            centers_ao, ls, anorms, coeffs, zetas, normalization, cart2sph)
        if not ok:
            raise RuntimeError("unexpected parameter structure")
        if "runner" not in _STATE:
            nc = build_module()
            _STATE["runner"] = _Runner(nc, N_CORES)
        runner = _STATE["runner"]
        outs = runner(_global_inputs(x, consts_np))
        return outs[0].astype(np.float32, copy=False)
    except Exception:
        import traceback
        traceback.print_exc()
        return _np_compute(x, centers_ao, ls, anorms, coeffs, zetas,
                           normalization, cart2sph)


# revision 3
# speedup vs baseline: 14816.1417x; 14816.1417x over previous
import numpy as np

# Problem (hardcoded from spec/reference):
#   x [131072,3]; per-cartesian-AO params: centers_ao [240,3], ls [240,3] int32,
#   anorms [240], coeffs/zetas [240,6], normalization [240], cart2sph [240,224].
#   Output [131072,224] float32.
#   Structure: 16 atoms x shells [s,s,s,p,p,d] -> 96 shells, 240 cart AOs,
#   6 primitives per shell. Per-AO arrays are duplicated per shell/atom.
#
# Strategy (8 NeuronCores, pure data parallel over points):
#   Host precomputes per-point features xf[26] = [1, x, y, z, xx, xy, xz, yy,
#   yz, zz, r2_atom0..r2_atom15] (r2 exact in fp32 on host). On device, in a
#   transposed layout (points along the free dim, F=512 per chunk):
#     m1 (PE, float32r): W1[26,816]^T @ xf -> [Arg(576) ; ang(240)] where
#         Arg[(s,p)] = -zeta[s,p]*r2[atom(s)] and ang[a] = cart angular poly.
#     exp (ACT):   E = exp(Arg) -> bf16 SBUF.
#     m2 (PE bf16): rad[96] = Coef[576,96]^T @ E  (contract 6 primitives).
#     m3 (PE bf16): rad_ao[240] = Expand[96,240]^T @ rad (shell->AO).
#     G (DVE): G = ang * rad_ao -> bf16.
#     m4 (PE bf16): out[128pts,224] = G[:,blk]^T @ C2 with C2 = diag(anorm*
#         normalization) @ cart2sph; output written per 128-point block in
#         row-major order, copied PSUM->SBUF on ACT, DMA'd to DRAM.
#   No collectives; each core computes its own 16384-point slice.

N_CORES = 8
N_POINTS = 131072
NPC = N_POINTS // N_CORES  # 16384
FCHUNK = 512
NATOM = 16
SHELL_LS_PER_ATOM = [0, 0, 0, 1, 1, 2]
NCART_OF_L = {0: 1, 1: 3, 2: 6}
NSH = NATOM * len(SHELL_LS_PER_ATOM)  # 96
NAO = 240
NSPH = 224
NPRIM = 6
NARG = NSH * NPRIM  # 576
NFEAT = 10 + NATOM  # 26
M1COLS = NARG + NAO  # 816

# AO/shell bookkeeping (reference order: per atom, per shell, per cartesian).
_AO_SHELL = []
_SHELL_ATOM = []
_s = 0
for _a in range(NATOM):
    for _l in SHELL_LS_PER_ATOM:
        _AO_SHELL.extend([_s] * NCART_OF_L[_l])
        _SHELL_ATOM.append(_a)
        _s += 1
_AO_SHELL = np.asarray(_AO_SHELL)
_SHELL_ATOM = np.asarray(_SHELL_ATOM)
_FIRST_AO_OF_SHELL = np.searchsorted(_AO_SHELL, np.arange(NSH))

_MON_IDX = {
    (): 0, (0,): 1, (1,): 2, (2,): 3,
    (0, 0): 4, (0, 1): 5, (0, 2): 6, (1, 1): 7, (1, 2): 8, (2, 2): 9,
}

_STATE = {}


def _expand_ao_poly(lvec, c):
    """Coefficients of prod_k (x_k - c_k)^l_k in the 10-monomial basis."""
    terms = {(): 1.0}
    for k in range(3):
        l = int(lvec[k])
        if l == 0:
            axis = {(): 1.0}
        elif l == 1:
            axis = {(k,): 1.0, (): -float(c[k])}
        elif l == 2:
            axis = {(k, k): 1.0, (k,): -2.0 * float(c[k]), (): float(c[k]) ** 2}
        else:
            raise ValueError(f"unsupported l={l}")
        new = {}
        for m1, c1 in terms.items():
            for m2, c2 in axis.items():
                m = tuple(sorted(m1 + m2))
                new[m] = new.get(m, 0.0) + c1 * c2
        terms = new
    return terms


def _build_constants(centers_ao, ls, anorms, coeffs, zetas, normalization, cart2sph):
    import ml_dtypes

    first_ao_atom = np.arange(NATOM) * (NAO // NATOM)
    centers_atom = centers_ao[first_ao_atom]              # [16,3]
    zetas_sh = zetas[_FIRST_AO_OF_SHELL]                  # [96,6]
    coeffs_sh = coeffs[_FIRST_AO_OF_SHELL]                # [96,6]

    # Sanity-check the assumed duplication structure; caller falls back if not.
    ok = (
        np.array_equal(zetas, zetas_sh[_AO_SHELL])
        and np.array_equal(coeffs, coeffs_sh[_AO_SHELL])
        and np.array_equal(centers_ao, centers_atom[_SHELL_ATOM[_AO_SHELL]])
        and int(ls.sum(axis=1).max()) <= 2
    )

    w1 = np.zeros((NFEAT, M1COLS), np.float32)
    for s in range(NSH):
        for p in range(NPRIM):
            w1[10 + s // 6, s * NPRIM + p] = -zetas_sh[s, p]
    for a in range(NAO):
        for mon, cf in _expand_ao_poly(ls[a], centers_ao[a]).items():
            w1[_MON_IDX[mon], NARG + a] = cf

    coefm = np.zeros((NARG, NSH), np.float32)
    for s in range(NSH):
        coefm[s * NPRIM:(s + 1) * NPRIM, s] = coeffs_sh[s]

    expm = np.zeros((NSH, NAO), np.float32)
    expm[_AO_SHELL, np.arange(NAO)] = 1.0

    c2 = (anorms * normalization)[:, None] * cart2sph     # [240,224]

    bf16 = ml_dtypes.bfloat16
    return ok, {
        "w1": w1,
        "coef": coefm.astype(bf16),
        "expm": expm.astype(bf16),
        "c2": c2.astype(bf16),
        "centers_atom": centers_atom,
    }


def _build_features(x, centers_atom):
    """xf [26, N]: [1, x, y, z, xx, xy, xz, yy, yz, zz, r2_0..r2_15]."""
    n = x.shape[0]
    xf = np.empty((NFEAT, n), np.float32)
    xf[0] = 1.0
    xf[1:4] = x.T
    xf[4] = x[:, 0] * x[:, 0]
    xf[5] = x[:, 0] * x[:, 1]
    xf[6] = x[:, 0] * x[:, 2]
    xf[7] = x[:, 1] * x[:, 1]
    xf[8] = x[:, 1] * x[:, 2]
    xf[9] = x[:, 2] * x[:, 2]
    dx = x[:, None, :] - centers_atom[None, :, :]
    xf[10:] = np.einsum("nak,nak->na", dx, dx).T
    return xf


def build_module(npc=NPC, fchunk=FCHUNK):
    """Build the per-core Bass/Tile module (same program on all cores)."""
    from contextlib import ExitStack

    import concourse.bass as bass
    import concourse.tile as tile
    from concourse import bacc, mybir

    dt = mybir.dt
    Exp = mybir.ActivationFunctionType.Exp
    F = fchunk
    nchunks = npc // F
    assert npc % F == 0 and F % 128 == 0

    nc = bacc.Bacc("TRN2", target_bir_lowering=False, debug=False)
    xf = nc.dram_tensor("xf", [NFEAT, npc], dt.float32r, kind="ExternalInput").ap()
    w1 = nc.dram_tensor("w1", [NFEAT, M1COLS], dt.float32r, kind="ExternalInput").ap()
    coef = nc.dram_tensor("coef", [NARG, NSH], dt.bfloat16, kind="ExternalInput").ap()
    expm = nc.dram_tensor("expm", [NSH, NAO], dt.bfloat16, kind="ExternalInput").ap()
    c2 = nc.dram_tensor("c2", [NAO, NSPH], dt.bfloat16, kind="ExternalInput").ap()
    out = nc.dram_tensor("out", [npc, NSPH], dt.float32, kind="ExternalOutput").ap()

    # m1 output column blocks: 5 Arg tiles then 2 ang tiles.
    col_blocks = [(0, 128), (128, 256), (256, 384), (384, 512), (512, 576),
                  (576, 704), (704, 816)]

    with tile.TileContext(nc) as tc, ExitStack() as ctx:
        consts = ctx.enter_context(tc.tile_pool(name="consts", bufs=1))
        xf_pool = ctx.enter_context(tc.tile_pool(name="xfp", bufs=3))
        e_pool = ctx.enter_context(tc.tile_pool(name="ep", bufs=10))
        ang_pool = ctx.enter_context(tc.tile_pool(name="angp", bufs=4))
        radsb_pool = ctx.enter_context(tc.tile_pool(name="radsbp", bufs=2))
        g_pool = ctx.enter_context(tc.tile_pool(name="gp", bufs=4))
        osb_pool = ctx.enter_context(tc.tile_pool(name="osbp", bufs=4))
        argang_ps = ctx.enter_context(tc.tile_pool(name="argangps", bufs=3, space="PSUM"))
        rad_ps = ctx.enter_context(tc.tile_pool(name="radps", bufs=1, space="PSUM"))
        radao_ps = ctx.enter_context(tc.tile_pool(name="radaops", bufs=2, space="PSUM"))
        out_ps = ctx.enter_context(tc.tile_pool(name="outps", bufs=2, space="PSUM"))

        w1_sb = consts.tile([NFEAT, M1COLS], dt.float32r, tag="w1")
        nc.sync.dma_start(out=w1_sb, in_=w1)
        coef_sb = []
        for t in range(5):
            m = min(128, NARG - t * 128)
            ct = consts.tile([m, NSH], dt.bfloat16, tag=f"coef{t}")
            nc.sync.dma_start(out=ct, in_=coef[t * 128:t * 128 + m, :])
            coef_sb.append((ct, m))
        expm_sb = consts.tile([NSH, NAO], dt.bfloat16, tag="expm")
        nc.sync.dma_start(out=expm_sb, in_=expm)
        c2_hi = consts.tile([128, NSPH], dt.bfloat16, tag="c2hi")
        nc.sync.dma_start(out=c2_hi, in_=c2[0:128, :])
        c2_lo = consts.tile([NAO - 128, NSPH], dt.bfloat16, tag="c2lo")
        nc.sync.dma_start(out=c2_lo, in_=c2[128:NAO, :])

        for ci in range(nchunks):
            xs = xf_pool.tile([NFEAT, F], dt.float32r, tag="xs")
            nc.sync.dma_start(out=xs, in_=xf[:, ci * F:(ci + 1) * F])

            e_list, ang_list = [], []
            for (c0, c1) in col_blocks:
                m = c1 - c0
                ps = argang_ps.tile([128, F], dt.float32, tag="argang")
                nc.tensor.matmul(out=ps[:m], lhsT=w1_sb[:, c0:c1], rhs=xs,
                                 start=True, stop=True)
                if c0 < NARG:
                    et = e_pool.tile([128, F], dt.bfloat16, tag="e")
                    nc.scalar.activation(out=et[:m], in_=ps[:m], func=Exp)
                    e_list.append((et, m))
                else:
                    at = ang_pool.tile([128, F], dt.bfloat16, tag="ang")
                    nc.vector.tensor_copy(out=at[:m], in_=ps[:m])
                    ang_list.append((at, m))

            rad = rad_ps.tile([NSH, F], dt.float32, tag="rad")
            for t, (et, m) in enumerate(e_list):
                nc.tensor.matmul(out=rad, lhsT=coef_sb[t][0], rhs=et[:m],
                                 start=(t == 0), stop=(t == len(e_list) - 1))
            rad_sb = radsb_pool.tile([NSH, F], dt.bfloat16, tag="radsb")
            nc.vector.tensor_copy(out=rad_sb, in_=rad)

            g_list = []
            for h, (a0, a1) in enumerate([(0, 128), (128, NAO)]):
                m = a1 - a0
                rao = radao_ps.tile([128, F], dt.float32, tag="radao")
                nc.tensor.matmul(out=rao[:m], lhsT=expm_sb[:, a0:a1], rhs=rad_sb,
                                 start=True, stop=True)
                gt = g_pool.tile([128, F], dt.bfloat16, tag="g")
                at, am = ang_list[h]
                assert am == m
                nc.vector.tensor_mul(gt[:m], rao[:m], at[:m])
                g_list.append((gt, m))

            for b in range(F // 128):
                ops = out_ps.tile([128, NSPH], dt.float32, tag="ops")
                nc.tensor.matmul(out=ops, lhsT=g_list[0][0][:, b * 128:(b + 1) * 128],
                                 rhs=c2_hi, start=True, stop=False)
                nc.tensor.matmul(out=ops,
                                 lhsT=g_list[1][0][:g_list[1][1], b * 128:(b + 1) * 128],
                                 rhs=c2_lo, start=False, stop=True)
                osb = osb_pool.tile([128, NSPH], dt.float32, tag="osb")
                nc.scalar.copy(out=osb, in_=ops)
                r0 = ci * F + b * 128
                nc.sync.dma_start(out=out[r0:r0 + 128, :], in_=osb)

    nc.compile()
    return nc


class _Runner:
    """Caches the jitted shard_map(bass_exec) across kernel() calls."""

    def __init__(self, nc, n_cores):
        import jax
        import concourse.mybir as mybir
        from concourse import bass2jax
        from jax.experimental.shard_map import shard_map
        from jax.sharding import Mesh, PartitionSpec

        bass2jax.install_neuronx_cc_hook()
        self.nc = nc
        self.n_cores = n_cores

        partition_name = (nc.partition_id_tensor.name
                          if nc.partition_id_tensor else None)
        in_names, out_names, out_avals = [], [], []
        for alloc in nc.m.functions[0].allocations:
            if not isinstance(alloc, mybir.MemoryLocationSet):
                continue
            name = alloc.memorylocations[0].name
            if alloc.kind == "ExternalInput":
                if name != partition_name:
                    in_names.append(name)
            elif alloc.kind == "ExternalOutput":
                out_names.append(name)
                out_avals.append(jax.core.ShapedArray(
                    tuple(alloc.tensor_shape), mybir.dt.np(alloc.dtype)))
        self.in_names = list(in_names)
        self.out_names = out_names
        self.out_avals = out_avals
        n_params = len(in_names)
        all_names = in_names + out_names
        if partition_name is not None:
            all_names = all_names + [partition_name]

        def _body(*args):
            operands = list(args)
            if partition_name is not None:
                operands.append(bass2jax.partition_id_tensor())
            outs = bass2jax._bass_exec_p.bind(
                *operands,
                out_avals=tuple(out_avals),
                in_names=tuple(all_names),
                out_names=tuple(out_names),
                lowering_input_output_aliases=(),
                sim_require_finite=True,
                sim_require_nnan=True,
                nc=nc,
            )
            return tuple(outs)

        devices = jax.devices()[:n_cores]
        assert len(devices) == n_cores
        mesh = Mesh(np.asarray(devices), ("core",))
        nargs = n_params + len(out_names)
        self._fn = jax.jit(
            shard_map(_body, mesh=mesh,
                      in_specs=(PartitionSpec("core"),) * nargs,
                      out_specs=(PartitionSpec("core"),) * len(out_names),
                      check_rep=False),
            donate_argnums=tuple(range(n_params, nargs)),
            keep_unused=True,
        )

    def __call__(self, global_inputs):
        """global_inputs: dict name -> [n_cores*dim0, ...] array. Returns
        list of global outputs (concatenated along axis 0)."""
        args = [global_inputs[name] for name in self.in_names]
        zeros = [np.zeros((self.n_cores * av.shape[0], *av.shape[1:]), av.dtype)
                 for av in self.out_avals]
        outs = self._fn(*args, *zeros)
        return [np.asarray(o) for o in outs]


def _np_compute(x, centers_ao, ls, anorms, coeffs, zetas, normalization, cart2sph):
    # CPU fallback, chunked over points to bound memory.
    N = x.shape[0]
    S = cart2sph.shape[1]
    out = np.empty((N, S), dtype=np.float32)
    w = (anorms * normalization).astype(np.float32)
    step = 8192
    for i in range(0, N, step):
        xb = x[i:i + step]
        dx = xb[:, None, :] - centers_ao[None, :, :]
        r2 = np.sum(dx * dx, axis=-1)
        ang = np.ones(r2.shape, dtype=np.float32)
        for k in range(3):
            d = dx[..., k]
            l = ls[None, :, k]
            ang = ang * np.where(l == 0, 1.0, np.where(l == 1, d, d * d)).astype(np.float32)
        rad = np.sum(coeffs[None] * np.exp(-zetas[None] * r2[..., None]), axis=-1)
        phi = (w[None] * ang * rad).astype(np.float32)
        out[i:i + step] = phi @ cart2sph
    return out


def _global_inputs(x, consts_np):
    """Assemble the concatenated per-core input arrays."""
    xf = _build_features(x, consts_np["centers_atom"])       # [26, N]
    xf_g = np.ascontiguousarray(
        xf.reshape(NFEAT, N_CORES, NPC).transpose(1, 0, 2).reshape(N_CORES * NFEAT, NPC))
    return {
        "xf": xf_g,
        "w1": np.tile(consts_np["w1"], (N_CORES, 1)),
        "coef": np.tile(consts_np["coef"], (N_CORES, 1)),
        "expm": np.tile(consts_np["expm"], (N_CORES, 1)),
        "c2": np.tile(consts_np["c2"], (N_CORES, 1)),
    }


def kernel(**inputs):
    x = np.asarray(inputs["x"], dtype=np.float32)
    centers_ao = np.asarray(inputs["centers_ao"], dtype=np.float32)
    ls = np.asarray(inputs["ls"], dtype=np.int32)
    anorms = np.asarray(inputs["anorms"], dtype=np.float32)
    coeffs = np.asarray(inputs["coeffs"], dtype=np.float32)
    zetas = np.asarray(inputs["zetas"], dtype=np.float32)
    normalization = np.asarray(inputs["normalization"], dtype=np.float32)
    cart2sph = np.asarray(inputs["cart2sph"], dtype=np.float32)

    try:
        if x.shape != (N_POINTS, 3) or cart2sph.shape != (NAO, NSPH):
            raise RuntimeError("unexpected shapes")
        ok, consts_np = _build_constants(
            centers_ao, ls, anorms, coeffs, zetas, normalization, cart2sph)
        if not ok:
            raise RuntimeError("unexpected parameter structure")
        if "runner" not in _STATE:
            nc = build_module()
            _STATE["runner"] = _Runner(nc, N_CORES)
        runner = _STATE["runner"]
        outs = runner(_global_inputs(x, consts_np))
        return outs[0].astype(np.float32, copy=False)
    except Exception:
        import traceback
        traceback.print_exc()
        return _np_compute(x, centers_ao, ls, anorms, coeffs, zetas,
                           normalization, cart2sph)
